# revision 43
# baseline (speedup 1.0000x reference)
"""AVSL similarity kernel for 8x trn2 NeuronCores (Bass/Tile).

Strategy
--------
Shard B1 (=256 query rows) across 8 cores, 32 rows each. All other inputs
are replicated. Everything else happens on-device per core:

Layout: "transposed" -- d (=512) on partitions (4 tiles of 128), B2 (=512)
on the free dim. This makes every per-(b1)-row quantity (e1n, alpha*cert1,
beta) a per-partition scalar, which ACT/DVE broadcast natively along the
free dim.

Per layer l:
  P^T    = Sigmoid(alpha*c1[b1] * c2^T + beta)   -- one ACT op/tile
  prop^T = W'^T_math @ nh^T                      -- PE, bf16, fp32 accum
  t      = (e2n^T - e1n[b1])^2 - prop            -- one fused custom DVE op
  nh^T   = P*t + prop                            -- mult (gpsimd/DVE) + add
Final: ovr[b1,:] = sum_e u2 + nh1 @ colsum(W2'): done on PE with M=32
one-hot selector weights so all 32 rows accumulate into one PSUM bank.

The main loop is software-pipelined with a 1-row skew: row b1's layer-1
matmuls are emitted alongside row (b1-1)'s layer-2 work so the PE never
waits on the elementwise tail of the current row.

Top-3 masking of links (per column) is done on-device in W^T layout via
3x reduce_max + scalar_tensor_tensor masking; colsum of the top-3 = m1+m2+m3.
"""

import sys

for _p in ("/opt/trn_rl_repo",):
    if _p not in sys.path:
        sys.path.insert(0, _p)

import numpy as np

import concourse.bass as bass
import concourse.bacc as bacc
import concourse.tile as tile
from concourse import mybir
from concourse import bass_utils
from concourse.masks import make_identity
import concourse.dve_ops as dve_ops
from concourse.dve_spec import Spec, Src0, Src1, C0, sq
from concourse.dve_table_gen import dve_ver_for


def _register_dve_op(name, spec):
    """Register a custom DVE op, self-pinning its uops sha."""
    import re
    for existing in dve_ops.OPS:
        if existing.name == name:
            return existing
    ver = dve_ver_for("TRN2")
    op = dve_ops.DveOp(name, spec, subdim=False, uops_sha={})
    dve_ops.OPS.append(op)
    dve_ops._SUB_OPCODE_FOR_NAME[name] = (
        dve_ops._CUSTOM_DVE_ROW_BASE + len(dve_ops.OPS) - 1)
    try:
        op.compile(ver)
    except ValueError as e:
        sha = re.search(r"v\d: ([0-9a-f]+)", str(e)).group(1)
        op = dve_ops.DveOp(name, spec, subdim=False, uops_sha={ver: sha})
        dve_ops.OPS[-1] = op
        dve_ops._COMPILE_CACHE.pop((name, ver), None)
    return op


# t = (e2n - e1n)^2 - prop in one DVE pass (in0=e2n, s0=-e1n, in1=prop)
TSUB_OP = _register_dve_op(
    "ANT_AVSL_TSUB",
    Spec(body=sq(Src0 + C0) - Src1,
         reference=lambda in0, in1, s0, s1, imm2:
             (in0.astype("float32") + s0) ** 2 - in1))

F32 = mybir.dt.float32
BF16 = mybir.dt.bfloat16
AF = mybir.ActivationFunctionType
ALU = mybir.AluOpType
AX = mybir.AxisListType.X

L = 3
B1 = 256
B2 = 512
D = 512
NCORES = 8
B1C = B1 // NCORES          # 32 rows per core
KT = D // 128               # 4 partition tiles of d (and of e, and of b2)
TOPK = 3
EPS_DIV = 1e-8


def _alloc4(pool, tag, dtype, shape=(128, B2), bufs=None):
    return [pool.tile(list(shape), dtype, tag=f"{tag}{k}", name=f"{tag}{k}",
                      bufs=bufs)
            for k in range(KT)]


def _transpose_tiles(nc, prep_ps, ident, nat_tiles, out_f=None, dt=F32):
    """Transpose natural tiles (p, 512) -> 4 tiles (128, p*n) via PE."""
    n_in = len(nat_tiles)
    p_in = nat_tiles[0].shape[0]
    for k in range(KT):
        ps = prep_ps.tile([128, p_in * n_in], dt, tag="tps", bufs=2,
                          name="tps")
        for j, nat in enumerate(nat_tiles):
            nc.tensor.transpose(
                ps[:, j * p_in:(j + 1) * p_in],
                nat[:, k * 128:(k + 1) * 128],
                ident[:p_in, :p_in],
            )
        out_f(k, ps)


def _rsqrt(nc, pool, ss, p, w=1):
    """r = 1/sqrt(ss), ss (p,w) f32. ACT sqrt seed + reciprocal + 2 Newton."""
    n = pool.tile([p, w], F32, tag="rs_n", name="rs_n")
    nc.scalar.activation(n[:], ss[:], AF.Sqrt)
    r = pool.tile([p, w], F32, tag="rs_r", name="rs_r")
    nc.vector.reciprocal(r[:], n[:])
    for it in range(1):
        a = pool.tile([p, w], F32, tag="rs_a", name="rs_a")
        nc.vector.tensor_mul(a[:], r[:], r[:])          # r^2
        b = pool.tile([p, w], F32, tag="rs_b", name="rs_b")
        nc.vector.tensor_mul(b[:], a[:], ss[:])         # ss*r^2
        c = pool.tile([p, w], F32, tag="rs_c", name="rs_c")
        nc.vector.tensor_scalar(c[:], b[:], -0.5, 1.5, ALU.mult, ALU.add)
        r2 = pool.tile([p, w], F32, tag="rs_r", name="rs_r")
        nc.vector.tensor_mul(r2[:], r[:], c[:])
        r = r2
    return r


def build_kernel(tc, io):
    nc = tc.nc
    emb1, cert1, emb2, cert2, links, alpha, beta, ovr_out = io

    import contextlib
    ctx = contextlib.ExitStack()
    with ctx:
        consts = ctx.enter_context(tc.tile_pool(name="consts", bufs=1))
        shared = ctx.enter_context(tc.tile_pool(name="shared", bufs=1))
        pctx = ctx.enter_context(contextlib.ExitStack())
        prep = pctx.enter_context(tc.tile_pool(name="prep", bufs=1))
        small = pctx.enter_context(tc.tile_pool(name="small", bufs=4))
        prep_ps = pctx.enter_context(
            tc.tile_pool(name="prep_ps", bufs=1, space="PSUM"))

        ident_f = consts.tile([128, 128], F32)
        make_identity(nc, ident_f[:])
        ident_b = consts.tile([128, 128], BF16)
        make_identity(nc, ident_b[:])

        # contiguous per-row selectors: zcat[:, b1*32:(b1+1)*32] has
        # one-hot column b1 (ones in every partition)
        zcat = consts.tile([128, 32 * B1C], BF16)
        nc.vector.memset(zcat[:], 0.0)
        for j in range(B1C):
            nc.gpsimd.affine_select(
                zcat[:, j * 32:(j + 1) * 32], zcat[:, j * 32:(j + 1) * 32],
                pattern=[[-1, 32]], compare_op=ALU.not_equal, fill=1.0,
                base=j, channel_multiplier=0)

        # ---------------- alpha / beta columns -----------------------------
        ab_nat = prep.tile([2, 512], F32, tag="abn")
        nc.sync.dma_start(ab_nat[:], alpha[:, :])
        bt_nat = prep.tile([2, 512], F32, tag="abn")
        nc.sync.dma_start(bt_nat[:], beta[:, :])
        alphaT = [consts.tile([128, 2], F32, tag=f"alT{k}", name=f"alT{k}")
                  for k in range(KT)]
        betaT = [consts.tile([128, 2], F32, tag=f"beT{k}", name=f"beT{k}")
                 for k in range(KT)]
        for src, dst in ((ab_nat, alphaT), (bt_nat, betaT)):
            for k in range(KT):
                ps = prep_ps.tile([128, 2], F32, tag="tps", bufs=2,
                                  name="tps")
                nc.tensor.transpose(
                    ps[:], src[:, k * 128:(k + 1) * 128], ident_f[:2, :2])
                nc.vector.tensor_copy(dst[k][:], ps[:])

        # ---------------- e2 normalize + transpose -------------------------
        e2nT = [[shared.tile([128, B2], BF16, tag=f"e2nT{l}{k}",
                             name=f"e2nT{l}{k}")
                 for k in range(KT)] for l in range(L)]
        for l in range(L):
            raw = []
            ss = small.tile([128, KT], F32, tag="ss", name="ss")
            for j in range(KT):
                nat = prep.tile([128, D], F32, tag=f"stg{j}",
                                name=f"stg{j}", bufs=2)
                nc.sync.dma_start(nat[:], emb2[l, j * 128:(j + 1) * 128, :])
                sqs = prep.tile([128, D], F32, tag="sqscratch",
                                name="sqscratch", bufs=1)
                nc.scalar.activation(sqs[:], nat[:], AF.Square,
                                     accum_out=ss[:, j:j + 1])
                raw.append(nat)
            r = _rsqrt(nc, small, ss, 128, KT)
            nats = []
            for j in range(KT):
                en = prep.tile([128, D], BF16, tag=f"e2n{j}",
                                name=f"e2n{j}", bufs=2)
                nc.vector.tensor_scalar_mul(en[:], raw[j][:], r[:, j:j + 1])
                nats.append(en)
            _transpose_tiles(
                nc, prep_ps, ident_b, nats, dt=BF16,
                out_f=lambda k, ps, l=l: nc.vector.tensor_copy(
                    e2nT[l][k][:], ps[:]))

        # ---------------- cert2 transpose (l=1,2) --------------------------
        cert2T = [None] + [[shared.tile([128, B2], BF16, tag=f"c2T{l}{k}",
                                        name=f"c2T{l}{k}")
                            for k in range(KT)] for l in (1, 2)]
        for l in (1, 2):
            nats = []
            for j in range(KT):
                nat = prep.tile([128, D], F32, tag=f"stg{j}",
                                name=f"stg{j}", bufs=2)
                nc.sync.dma_start(nat[:], cert2[l, j * 128:(j + 1) * 128, :])
                nats.append(nat)
            _transpose_tiles(
                nc, prep_ps, ident_f, nats,
                out_f=lambda k, ps, l=l: nc.vector.tensor_copy(
                    cert2T[l][k][:], ps[:]))

        # ---------------- e1 normalize + transpose (negated) ---------------
        neg_e1T = [[consts.tile([128, B1C], F32, tag=f"ne1T{l}{k}",
                                name=f"ne1T{l}{k}")
                    for k in range(KT)] for l in range(L)]
        for l in range(L):
            nat = prep.tile([B1C, D], F32, tag="e1nat")
            nc.sync.dma_start(nat[:], emb1[l, :, :])
            sqs = prep.tile([B1C, D], F32, tag="sq1scratch")
            ss = small.tile([B1C, 1], F32, tag="ss1")
            nc.scalar.activation(sqs[:], nat[:], AF.Square, accum_out=ss[:])
            r = _rsqrt(nc, small, ss, B1C)
            # fold the negation into the normalize scale: -e1n = nat * (-r)
            rneg = small.tile([B1C, 1], F32, tag="rneg")
            nc.vector.tensor_scalar_mul(rneg[:], r[:], -1.0)
            en = prep.tile([B1C, D], F32, tag="e1n")
            nc.vector.tensor_scalar_mul(en[:], nat[:], rneg[:])
            _transpose_tiles(
                nc, prep_ps, ident_f, [en],
                out_f=lambda k, ps, l=l: nc.vector.tensor_copy(
                    neg_e1T[l][k][:], ps[:]))

        # ---------------- cert1 -> alpha * cert1^T (l=1,2) -----------------
        aC1T = [None, [], []]
        for l in (1, 2):
            nat = prep.tile([B1C, D], F32, tag="c1nat")
            nc.sync.dma_start(nat[:], cert1[l, :, :])

            def mk(k, ps, l=l):
                t = consts.tile([128, B1C], F32, tag=f"ac1T{l}{k}",
                                name=f"ac1T{l}{k}")
                nc.vector.tensor_scalar_mul(t[:], ps[:], alphaT[k][:, l - 1:l])
                aC1T[l].append(t)
            _transpose_tiles(nc, prep_ps, ident_f, [nat], out_f=mk)

        # ---------------- links: topk mask + normalize + transpose ---------
        Wp = [None, [], []]   # W' natural, bf16, per layer: 4 tiles (128d, 512e)
        w2stair = []          # selector tiles carrying colsum(W2') at col b1
        for l in (1, 2):
            nats = []
            for j in range(KT):
                nat = prep.tile([128, D], F32, tag=f"stg{j}",
                                name=f"stg{j}", bufs=2)
                nc.sync.dma_start(nat[:], links[l - 1, j * 128:(j + 1) * 128, :])
                nats.append(nat)
            WT = []

            def stash(k, ps):
                t = prep.tile([128, D], F32, tag=f"wT{k}", name=f"wT{k}")
                nc.vector.tensor_copy(t[:], ps[:])
                WT.append(t)
            _transpose_tiles(nc, prep_ps, ident_f, nats, out_f=stash)

            WpT = []
            for k in range(KT):
                m1 = small.tile([128, 1], F32, tag="m1")
                nc.vector.tensor_reduce(m1[:], WT[k][:], axis=AX, op=ALU.max)
                w1 = prep.tile([128, D], F32, tag="wtmp", name="w1", bufs=2)
                nc.vector.scalar_tensor_tensor(
                    w1[:], WT[k][:], m1[:], WT[k][:], ALU.is_lt, ALU.mult)
                m2 = small.tile([128, 1], F32, tag="m2")
                nc.vector.tensor_reduce(m2[:], w1[:], axis=AX, op=ALU.max)
                w2 = prep.tile([128, D], F32, tag="wtmp", name="w2", bufs=2)
                nc.vector.scalar_tensor_tensor(
                    w2[:], w1[:], m2[:], w1[:], ALU.is_lt, ALU.mult)
                m3 = small.tile([128, 1], F32, tag="m3")
                nc.vector.tensor_reduce(m3[:], w2[:], axis=AX, op=ALU.max)
                # colsum of kept top-3 = m1+m2+m3 (+eps); scale = 1/that
                cs = small.tile([128, 1], F32, tag="cs")
                nc.vector.tensor_add(cs[:], m1[:], m2[:])
                cs2 = small.tile([128, 1], F32, tag="cs2")
                nc.vector.tensor_add(cs2[:], cs[:], m3[:])
                cs3 = small.tile([128, 1], F32, tag="cs3")
                nc.vector.tensor_scalar_add(cs3[:], cs2[:], EPS_DIV)
                rcs = small.tile([128, 1], F32, tag="rcs")
                nc.vector.reciprocal(rcs[:], cs3[:])
                wm = prep.tile([128, D], F32, tag="wtmp", name="wm", bufs=2)
                nc.vector.scalar_tensor_tensor(
                    wm[:], WT[k][:], m3[:], WT[k][:], ALU.is_ge, ALU.mult)
                wpt = prep.tile([128, D], BF16, tag=f"wpT{k}", name=f"wpT{k}")
                nc.vector.tensor_scalar_mul(wpt[:], wm[:], rcs[:])
                WpT.append(wpt)

            # transpose back to natural (d on partitions) in bf16
            for j in range(KT):
                ps = prep_ps.tile([128, D], BF16, tag="tps", bufs=2,
                                  name="tps")
                for k in range(KT):
                    nc.tensor.transpose(
                        ps[:, k * 128:(k + 1) * 128],
                        WpT[k][:, j * 128:(j + 1) * 128],
                        ident_b[:])
                wn = shared.tile([128, D], BF16, tag=f"wp{l}{j}",
                                 name=f"wp{l}{j}")
                nc.vector.tensor_copy(wn[:], ps[:])
                Wp[l].append(wn)

            if l == 2:
                for j in range(KT):
                    w2s = small.tile([128, 1], F32, tag="w2s")
                    nc.vector.tensor_reduce(
                        w2s[:], Wp[2][j][:], axis=AX, op=ALU.add)
                    st = consts.tile([128, 63], BF16, tag=f"w2st{j}",
                                     name=f"w2st{j}")
                    nc.vector.memset(st[:], 0.0)
                    nc.vector.tensor_copy(st[:, 31:32], w2s[:])
                    w2stair.append(st)

        # ======================= main loop over b1 ==========================
        # Software-pipelined: emit (b1, layer1) and (b1-1, layer2) together.
        mp = ctx.enter_context(tc.tile_pool(name="main", bufs=3))
        pp = ctx.enter_context(tc.tile_pool(name="mps", bufs=5, space="PSUM"))
        pov = ctx.enter_context(tc.tile_pool(name="ovrps", bufs=1,
                                             space="PSUM"))
        ovr_ps = pov.tile([B1C, B2], F32, tag="ovr")
        N_RED_TOTAL = B1C * 2 * KT
        nh_of = {}

        def emit_l0_l1_pair(b1a, b1b):
            nh0_of, pg_of, nh_new = {}, {}, {}
            for b1 in (b1a, b1b):
                nh0 = _alloc4(mp, "n0", BF16)
                for k in range(KT):
                    nc.scalar.activation(
                        nh0[k][:], e2nT[0][k][:], AF.Square,
                        bias=neg_e1T[0][k][:, b1:b1 + 1])
                Pg = _alloc4(mp, "pg", BF16)
                for k in range(KT):
                    nc.scalar.activation(
                        Pg[k][:], cert2T[1][k][:], AF.Sigmoid,
                        bias=betaT[k][:, 0:1],
                        scale=aC1T[1][k][:, b1:b1 + 1])
                nh0_of[b1] = nh0
                pg_of[b1] = Pg
                nh_new[b1] = _alloc4(mp, "nh", BF16, bufs=4)
            for e in range(KT):
                prs = {}
                for b1 in (b1a, b1b):
                    prs[b1] = pp.tile([128, B2], F32, tag="prop", name="prop")
                for k in range(KT):
                    for b1 in (b1a, b1b):
                        nc.tensor.matmul(
                            prs[b1][:],
                            lhsT=Wp[1][k][:, e * 128:(e + 1) * 128],
                            rhs=nh0_of[b1][k][:],
                            start=(k == 0), stop=(k == KT - 1))
                for b1 in (b1a, b1b):
                    pr = prs[b1]
                    t = mp.tile([128, B2], BF16, tag=f"t{e}", bufs=4,
                                name=f"t{e}")
                    nc.vector._custom_dve(
                        TSUB_OP, out=t[:], in0=e2nT[1][e][:], in1=pr[:],
                        s0=neg_e1T[1][e][:, b1:b1 + 1])
                    u = mp.tile([128, B2], BF16, tag=f"u{e}", bufs=4,
                                name=f"u{e}")
                    if (e + (0 if b1 == b1a else 1)) % 2 == 0:
                        nc.gpsimd.tensor_mul(u[:], pg_of[b1][e][:], t[:])
                    else:
                        nc.vector.tensor_mul(u[:], pg_of[b1][e][:], t[:])
                    nc.vector.tensor_add(nh_new[b1][e][:], u[:], pr[:])
            nh_of[b1a] = nh_new[b1a]
            nh_of[b1b] = nh_new[b1b]

        def emit_l2(b1, n_red):
            nh1 = nh_of.pop(b1)
            Pg = _alloc4(mp, "qg", BF16)
            for k in range(KT):
                nc.scalar.activation(
                    Pg[k][:], cert2T[2][k][:], AF.Sigmoid,
                    bias=betaT[k][:, 1:2],
                    scale=aC1T[2][k][:, b1:b1 + 1])
            prs = []
            for e in range(KT):
                pr = pp.tile([128, B2], F32, tag="prop", name="prop")
                for k in range(KT):
                    nc.tensor.matmul(
                        pr[:],
                        lhsT=Wp[2][k][:, e * 128:(e + 1) * 128],
                        rhs=nh1[k][:],
                        start=(k == 0), stop=(k == KT - 1))
                prs.append(pr)
            for e in range(KT):
                pr = prs[e]
                t = mp.tile([128, B2], BF16, tag=f"s{e}", bufs=3,
                            name=f"s{e}")
                nc.vector._custom_dve(
                    TSUB_OP, out=t[:], in0=e2nT[2][e][:], in1=pr[:],
                    s0=neg_e1T[2][e][:, b1:b1 + 1])
                u = mp.tile([128, B2], BF16, tag=f"v{e}", bufs=3,
                            name=f"v{e}")
                if e % 2 == 0:
                    nc.gpsimd.tensor_mul(u[:], Pg[e][:], t[:])
                else:
                    nc.vector.tensor_mul(u[:], Pg[e][:], t[:])
                nc.tensor.matmul(
                    ovr_ps[:],
                    lhsT=zcat[:, b1 * 32:(b1 + 1) * 32],
                    rhs=u[:],
                    start=(n_red == 0), stop=False,
                    skip_group_check=True)
                n_red += 1
            for k in range(KT):
                nc.tensor.matmul(
                    ovr_ps[:],
                    lhsT=w2stair[k][:, 31 - b1:63 - b1],
                    rhs=nh1[k][:],
                    start=False, stop=(n_red == N_RED_TOTAL - 1),
                    skip_group_check=True)
                n_red += 1
            return n_red

        n_red = 0
        NPAIR = B1C // 2
        for p in range(NPAIR + 1):
            if p < NPAIR:
                emit_l0_l1_pair(2 * p, 2 * p + 1)
            if p >= 1:
                n_red = emit_l2(2 * (p - 1), n_red)
                n_red = emit_l2(2 * (p - 1) + 1, n_red)

        ovr_sb = mp.tile([B1C, B2], F32, tag="ovr_sb", bufs=1)
        nc.vector.tensor_copy(ovr_sb[:], ovr_ps[:])
        nc.sync.dma_start(ovr_out[:, :], ovr_sb[:])


_CACHED = None


def _build():
    global _CACHED
    if _CACHED is not None:
        return _CACHED
    nc = bacc.Bacc(
        "TRN2", target_bir_lowering=False, debug=False,
        enable_asserts=False, num_devices=NCORES)
    io = (
        nc.dram_tensor("emb1", (L, B1C, D), F32, kind="ExternalInput").ap(),
        nc.dram_tensor("cert1", (L, B1C, D), F32, kind="ExternalInput").ap(),
        nc.dram_tensor("emb2", (L, B2, D), F32, kind="ExternalInput").ap(),
        nc.dram_tensor("cert2", (L, B2, D), F32, kind="ExternalInput").ap(),
        nc.dram_tensor("links", (L - 1, D, D), F32, kind="ExternalInput").ap(),
        nc.dram_tensor("alpha", (L - 1, D), F32, kind="ExternalInput").ap(),
        nc.dram_tensor("beta", (L - 1, D), F32, kind="ExternalInput").ap(),
        nc.dram_tensor("ovr", (B1C, B2), F32, kind="ExternalOutput").ap(),
    )
    with tile.TileContext(nc) as tc:
        build_kernel(tc, io)
    nc.compile()
    _CACHED = nc
    return nc


def _run(inputs, trace=False, **kw):
    nc = _build()
    arr = {k: np.ascontiguousarray(np.asarray(v, dtype=np.float32))
           for k, v in inputs.items()}
    in_maps = []
    for c in range(NCORES):
        sl = slice(c * B1C, (c + 1) * B1C)
        in_maps.append({
            "emb1": arr["emb1"][:, sl, :],
            "cert1": arr["cert1"][:, sl, :],
            "emb2": arr["emb2"],
            "cert2": arr["cert2"],
            "links": arr["links"],
            "alpha": arr["alpha"],
            "beta": arr["beta"],
        })
    res = bass_utils.run_bass_kernel_spmd(
        nc, in_maps, core_ids=list(range(NCORES)), trace=trace, **kw)
    out = np.concatenate([res.results[c]["ovr"] for c in range(NCORES)],
                         axis=0)
    return out, res


def kernel(**inputs) -> np.ndarray:
    try:
        out, _ = _run(inputs, trace=False)
    except Exception:
        # transient device failures (e.g. NRT_EXEC_UNIT_UNRECOVERABLE):
        # retry once on a fresh run
        import time
        time.sleep(5)
        out, _ = _run(inputs, trace=False)
    return out.astype(np.float32)


if __name__ == "__main__":
    nc = _build()
    print("build+compile OK; instructions:",
          sum(len(b.instructions) for f in nc.m.functions for b in f.blocks))


# revision 44
# speedup vs baseline: 1.1563x; 1.1563x over previous
"""AVSL similarity kernel for 8x trn2 NeuronCores (Bass/Tile).

Strategy
--------
Shard B1 (=256 query rows) across 8 cores, 32 rows each. All other inputs
are replicated. Everything else happens on-device per core:

Layout: "transposed" -- d (=512) on partitions (4 tiles of 128), B2 (=512)
on the free dim. This makes every per-(b1)-row quantity (e1n, alpha*cert1,
beta) a per-partition scalar, which ACT/DVE broadcast natively along the
free dim.

Per layer l:
  P^T    = Sigmoid(alpha*c1[b1] * c2^T + beta)   -- one ACT op/tile
  prop^T = W'^T_math @ nh^T                      -- PE, bf16, fp32 accum
  t      = (e2n^T - e1n[b1])^2 - prop            -- one fused custom DVE op
  nh^T   = P*t + prop                            -- mult (gpsimd/DVE) + add
Final: ovr[b1,:] = sum_e u2 + nh1 @ colsum(W2'): done on PE with M=32
one-hot selector weights so all 32 rows accumulate into one PSUM bank.

The main loop is software-pipelined with a 1-row skew: row b1's layer-1
matmuls are emitted alongside row (b1-1)'s layer-2 work so the PE never
waits on the elementwise tail of the current row.

Top-3 masking of links (per column) is done on-device in W^T layout via
3x reduce_max + scalar_tensor_tensor masking; colsum of the top-3 = m1+m2+m3.
"""

import sys

for _p in ("/opt/trn_rl_repo",):
    if _p not in sys.path:
        sys.path.insert(0, _p)

import numpy as np

import concourse.bass as bass
import concourse.bacc as bacc
import concourse.tile as tile
from concourse import mybir
from concourse import bass_utils
from concourse.masks import make_identity
import concourse.dve_ops as dve_ops
from concourse.dve_spec import Spec, Src0, Src1, C0, sq
from concourse.dve_table_gen import dve_ver_for


def _register_dve_op(name, spec):
    """Register a custom DVE op, self-pinning its uops sha."""
    import re
    for existing in dve_ops.OPS:
        if existing.name == name:
            return existing
    ver = dve_ver_for("TRN2")
    op = dve_ops.DveOp(name, spec, subdim=False, uops_sha={})
    dve_ops.OPS.append(op)
    dve_ops._SUB_OPCODE_FOR_NAME[name] = (
        dve_ops._CUSTOM_DVE_ROW_BASE + len(dve_ops.OPS) - 1)
    try:
        op.compile(ver)
    except ValueError as e:
        sha = re.search(r"v\d: ([0-9a-f]+)", str(e)).group(1)
        op = dve_ops.DveOp(name, spec, subdim=False, uops_sha={ver: sha})
        dve_ops.OPS[-1] = op
        dve_ops._COMPILE_CACHE.pop((name, ver), None)
    return op


# t = (e2n - e1n)^2 - prop in one DVE pass (in0=e2n, s0=-e1n, in1=prop)
TSUB_OP = _register_dve_op(
    "ANT_AVSL_TSUB",
    Spec(body=sq(Src0 + C0) - Src1,
         reference=lambda in0, in1, s0, s1, imm2:
             (in0.astype("float32") + s0) ** 2 - in1))

F32 = mybir.dt.float32
BF16 = mybir.dt.bfloat16
AF = mybir.ActivationFunctionType
ALU = mybir.AluOpType
AX = mybir.AxisListType.X

L = 3
B1 = 256
B2 = 512
D = 512
NCORES = 8
B1C = B1 // NCORES          # 32 rows per core
KT = D // 128               # 4 partition tiles of d (and of e, and of b2)
TOPK = 3
EPS_DIV = 1e-8


def _alloc4(pool, tag, dtype, shape=(128, B2), bufs=None):
    return [pool.tile(list(shape), dtype, tag=f"{tag}{k}", name=f"{tag}{k}",
                      bufs=bufs)
            for k in range(KT)]


def _transpose_tiles(nc, prep_ps, ident, nat_tiles, out_f=None, dt=F32):
    """Transpose natural tiles (p, 512) -> 4 tiles (128, p*n) via PE."""
    n_in = len(nat_tiles)
    p_in = nat_tiles[0].shape[0]
    for k in range(KT):
        ps = prep_ps.tile([128, p_in * n_in], dt, tag="tps", bufs=2,
                          name="tps")
        for j, nat in enumerate(nat_tiles):
            nc.tensor.transpose(
                ps[:, j * p_in:(j + 1) * p_in],
                nat[:, k * 128:(k + 1) * 128],
                ident[:p_in, :p_in],
            )
        out_f(k, ps)


def _rsqrt(nc, pool, ss, p, w=1):
    """r = 1/sqrt(ss), ss (p,w) f32. ACT sqrt seed + reciprocal + 2 Newton."""
    n = pool.tile([p, w], F32, tag="rs_n", name="rs_n")
    nc.scalar.activation(n[:], ss[:], AF.Sqrt)
    r = pool.tile([p, w], F32, tag="rs_r", name="rs_r")
    nc.vector.reciprocal(r[:], n[:])
    for it in range(1):
        a = pool.tile([p, w], F32, tag="rs_a", name="rs_a")
        nc.vector.tensor_mul(a[:], r[:], r[:])          # r^2
        b = pool.tile([p, w], F32, tag="rs_b", name="rs_b")
        nc.vector.tensor_mul(b[:], a[:], ss[:])         # ss*r^2
        c = pool.tile([p, w], F32, tag="rs_c", name="rs_c")
        nc.vector.tensor_scalar(c[:], b[:], -0.5, 1.5, ALU.mult, ALU.add)
        r2 = pool.tile([p, w], F32, tag="rs_r", name="rs_r")
        nc.vector.tensor_mul(r2[:], r[:], c[:])
        r = r2
    return r


def build_kernel(tc, io):
    nc = tc.nc
    emb1, cert1, emb2, cert2, links, alpha, beta, ovr_out = io

    import contextlib
    ctx = contextlib.ExitStack()
    with ctx:
        consts = ctx.enter_context(tc.tile_pool(name="consts", bufs=1))
        shared = ctx.enter_context(tc.tile_pool(name="shared", bufs=1))
        pctx = ctx.enter_context(contextlib.ExitStack())
        prep = pctx.enter_context(tc.tile_pool(name="prep", bufs=1))
        small = pctx.enter_context(tc.tile_pool(name="small", bufs=4))
        prep_ps = pctx.enter_context(
            tc.tile_pool(name="prep_ps", bufs=1, space="PSUM"))

        ident_f = consts.tile([128, 128], F32)
        make_identity(nc, ident_f[:])
        ident_b = consts.tile([128, 128], BF16)
        make_identity(nc, ident_b[:])

        # contiguous per-row selectors: zcat[:, b1*32:(b1+1)*32] has
        # one-hot column b1 (ones in every partition)
        zcat = consts.tile([128, 32 * B1C], BF16)
        nc.vector.memset(zcat[:], 0.0)
        for j in range(B1C):
            nc.gpsimd.affine_select(
                zcat[:, j * 32:(j + 1) * 32], zcat[:, j * 32:(j + 1) * 32],
                pattern=[[-1, 32]], compare_op=ALU.not_equal, fill=1.0,
                base=j, channel_multiplier=0)

        # ---------------- alpha / beta columns -----------------------------
        ab_nat = prep.tile([2, 512], F32, tag="abn")
        nc.sync.dma_start(ab_nat[:], alpha[:, :])
        bt_nat = prep.tile([2, 512], F32, tag="abn")
        nc.sync.dma_start(bt_nat[:], beta[:, :])
        alphaT = [consts.tile([128, 2], F32, tag=f"alT{k}", name=f"alT{k}")
                  for k in range(KT)]
        betaT = [consts.tile([128, 2], F32, tag=f"beT{k}", name=f"beT{k}")
                 for k in range(KT)]
        for src, dst in ((ab_nat, alphaT), (bt_nat, betaT)):
            for k in range(KT):
                ps = prep_ps.tile([128, 2], F32, tag="tps", bufs=2,
                                  name="tps")
                nc.tensor.transpose(
                    ps[:], src[:, k * 128:(k + 1) * 128], ident_f[:2, :2])
                nc.vector.tensor_copy(dst[k][:], ps[:])

        # ---------------- e2 normalize + transpose -------------------------
        e2nT = [[shared.tile([128, B2], BF16, tag=f"e2nT{l}{k}",
                             name=f"e2nT{l}{k}")
                 for k in range(KT)] for l in range(L)]
        for l in range(L):
            raw = []
            ss = small.tile([128, KT], F32, tag="ss", name="ss")
            for j in range(KT):
                nat = prep.tile([128, D], F32, tag=f"stg{j}",
                                name=f"stg{j}", bufs=2)
                nc.sync.dma_start(nat[:], emb2[l, j * 128:(j + 1) * 128, :])
                sqs = prep.tile([128, D], F32, tag="sqscratch",
                                name="sqscratch", bufs=1)
                nc.scalar.activation(sqs[:], nat[:], AF.Square,
                                     accum_out=ss[:, j:j + 1])
                raw.append(nat)
            r = _rsqrt(nc, small, ss, 128, KT)
            nats = []
            for j in range(KT):
                en = prep.tile([128, D], BF16, tag=f"e2n{j}",
                                name=f"e2n{j}", bufs=2)
                nc.vector.tensor_scalar_mul(en[:], raw[j][:], r[:, j:j + 1])
                nats.append(en)
            _transpose_tiles(
                nc, prep_ps, ident_b, nats, dt=BF16,
                out_f=lambda k, ps, l=l: nc.vector.tensor_copy(
                    e2nT[l][k][:], ps[:]))

        # ---------------- cert2 transpose (l=1,2) --------------------------
        cert2T = [None] + [[shared.tile([128, B2], BF16, tag=f"c2T{l}{k}",
                                        name=f"c2T{l}{k}")
                            for k in range(KT)] for l in (1, 2)]
        for l in (1, 2):
            nats = []
            for j in range(KT):
                nat = prep.tile([128, D], F32, tag=f"stg{j}",
                                name=f"stg{j}", bufs=2)
                nc.sync.dma_start(nat[:], cert2[l, j * 128:(j + 1) * 128, :])
                nats.append(nat)
            _transpose_tiles(
                nc, prep_ps, ident_f, nats,
                out_f=lambda k, ps, l=l: nc.vector.tensor_copy(
                    cert2T[l][k][:], ps[:]))

        # ---------------- e1 normalize + transpose (negated) ---------------
        neg_e1T = [[consts.tile([128, B1C], F32, tag=f"ne1T{l}{k}",
                                name=f"ne1T{l}{k}")
                    for k in range(KT)] for l in range(L)]
        for l in range(L):
            nat = prep.tile([B1C, D], F32, tag="e1nat")
            nc.sync.dma_start(nat[:], emb1[l, :, :])
            sqs = prep.tile([B1C, D], F32, tag="sq1scratch")
            ss = small.tile([B1C, 1], F32, tag="ss1")
            nc.scalar.activation(sqs[:], nat[:], AF.Square, accum_out=ss[:])
            r = _rsqrt(nc, small, ss, B1C)
            # fold the negation into the normalize scale: -e1n = nat * (-r)
            rneg = small.tile([B1C, 1], F32, tag="rneg")
            nc.vector.tensor_scalar_mul(rneg[:], r[:], -1.0)
            en = prep.tile([B1C, D], F32, tag="e1n")
            nc.vector.tensor_scalar_mul(en[:], nat[:], rneg[:])
            _transpose_tiles(
                nc, prep_ps, ident_f, [en],
                out_f=lambda k, ps, l=l: nc.vector.tensor_copy(
                    neg_e1T[l][k][:], ps[:]))

        # ---------------- cert1 -> alpha * cert1^T (l=1,2) -----------------
        aC1T = [None, [], []]
        for l in (1, 2):
            nat = prep.tile([B1C, D], F32, tag="c1nat")
            nc.sync.dma_start(nat[:], cert1[l, :, :])

            def mk(k, ps, l=l):
                t = consts.tile([128, B1C], F32, tag=f"ac1T{l}{k}",
                                name=f"ac1T{l}{k}")
                nc.vector.tensor_scalar_mul(t[:], ps[:], alphaT[k][:, l - 1:l])
                aC1T[l].append(t)
            _transpose_tiles(nc, prep_ps, ident_f, [nat], out_f=mk)

        # ---------------- links: topk mask + normalize + transpose ---------
        Wp = [None, [], []]   # W' natural, bf16, per layer: 4 tiles (128d, 512e)
        w2stair = []          # selector tiles carrying colsum(W2') at col b1
        for l in (1, 2):
            nats = []
            for j in range(KT):
                nat = prep.tile([128, D], F32, tag=f"stg{j}",
                                name=f"stg{j}", bufs=2)
                nc.sync.dma_start(nat[:], links[l - 1, j * 128:(j + 1) * 128, :])
                nats.append(nat)
            WT = []

            def stash(k, ps):
                t = prep.tile([128, D], F32, tag=f"wT{k}", name=f"wT{k}")
                nc.vector.tensor_copy(t[:], ps[:])
                WT.append(t)
            _transpose_tiles(nc, prep_ps, ident_f, nats, out_f=stash)

            WpT = []
            for k in range(KT):
                m1 = small.tile([128, 1], F32, tag="m1")
                nc.vector.tensor_reduce(m1[:], WT[k][:], axis=AX, op=ALU.max)
                w1 = prep.tile([128, D], F32, tag="wtmp", name="w1", bufs=2)
                nc.vector.scalar_tensor_tensor(
                    w1[:], WT[k][:], m1[:], WT[k][:], ALU.is_lt, ALU.mult)
                m2 = small.tile([128, 1], F32, tag="m2")
                nc.vector.tensor_reduce(m2[:], w1[:], axis=AX, op=ALU.max)
                w2 = prep.tile([128, D], F32, tag="wtmp", name="w2", bufs=2)
                nc.vector.scalar_tensor_tensor(
                    w2[:], w1[:], m2[:], w1[:], ALU.is_lt, ALU.mult)
                m3 = small.tile([128, 1], F32, tag="m3")
                nc.vector.tensor_reduce(m3[:], w2[:], axis=AX, op=ALU.max)
                # colsum of kept top-3 = m1+m2+m3 (+eps); scale = 1/that
                cs = small.tile([128, 1], F32, tag="cs")
                nc.vector.tensor_add(cs[:], m1[:], m2[:])
                cs2 = small.tile([128, 1], F32, tag="cs2")
                nc.vector.tensor_add(cs2[:], cs[:], m3[:])
                cs3 = small.tile([128, 1], F32, tag="cs3")
                nc.vector.tensor_scalar_add(cs3[:], cs2[:], EPS_DIV)
                rcs = small.tile([128, 1], F32, tag="rcs")
                nc.vector.reciprocal(rcs[:], cs3[:])
                wm = prep.tile([128, D], F32, tag="wtmp", name="wm", bufs=2)
                nc.vector.scalar_tensor_tensor(
                    wm[:], WT[k][:], m3[:], WT[k][:], ALU.is_ge, ALU.mult)
                wpt = prep.tile([128, D], BF16, tag=f"wpT{k}", name=f"wpT{k}")
                nc.vector.tensor_scalar_mul(wpt[:], wm[:], rcs[:])
                WpT.append(wpt)

            # transpose back to natural (d on partitions) in bf16
            for j in range(KT):
                ps = prep_ps.tile([128, D], BF16, tag="tps", bufs=2,
                                  name="tps")
                for k in range(KT):
                    nc.tensor.transpose(
                        ps[:, k * 128:(k + 1) * 128],
                        WpT[k][:, j * 128:(j + 1) * 128],
                        ident_b[:])
                wn = shared.tile([128, D], BF16, tag=f"wp{l}{j}",
                                 name=f"wp{l}{j}")
                nc.vector.tensor_copy(wn[:], ps[:])
                Wp[l].append(wn)

            if l == 2:
                for j in range(KT):
                    w2s = small.tile([128, 1], F32, tag="w2s")
                    nc.vector.tensor_reduce(
                        w2s[:], Wp[2][j][:], axis=AX, op=ALU.add)
                    st = consts.tile([128, 63], BF16, tag=f"w2st{j}",
                                     name=f"w2st{j}")
                    nc.vector.memset(st[:], 0.0)
                    nc.vector.tensor_copy(st[:, 31:32], w2s[:])
                    w2stair.append(st)

        # ======================= main loop over b1 ==========================
        # Software-pipelined: emit (b1, layer1) and (b1-1, layer2) together.
        mp = ctx.enter_context(tc.tile_pool(name="main", bufs=3))
        pp = ctx.enter_context(tc.tile_pool(name="mps", bufs=5, space="PSUM"))
        pov = ctx.enter_context(tc.tile_pool(name="ovrps", bufs=1,
                                             space="PSUM"))
        ovr_ps = pov.tile([B1C, B2], F32, tag="ovr")
        N_RED_TOTAL = B1C * 2 * KT
        nh_of = {}

        def emit_l0_l1(b1):
            nh0 = _alloc4(mp, "n0", BF16)
            for k in range(KT):
                nc.scalar.activation(
                    nh0[k][:], e2nT[0][k][:], AF.Square,
                    bias=neg_e1T[0][k][:, b1:b1 + 1])
            Pg = _alloc4(mp, "pg", BF16)
            for k in range(KT):
                nc.scalar.activation(
                    Pg[k][:], cert2T[1][k][:], AF.Sigmoid,
                    bias=betaT[k][:, 0:1],
                    scale=aC1T[1][k][:, b1:b1 + 1])
            nh = _alloc4(mp, "nh", BF16, bufs=4)
            prs = []
            for e in range(KT):
                pr = pp.tile([128, B2], F32, tag="prop", name="prop")
                for k in range(KT):
                    nc.tensor.matmul(
                        pr[:],
                        lhsT=Wp[1][k][:, e * 128:(e + 1) * 128],
                        rhs=nh0[k][:],
                        start=(k == 0), stop=(k == KT - 1))
                prs.append(pr)
            for e in range(KT):
                pr = prs[e]
                t = mp.tile([128, B2], BF16, tag=f"t{e}", bufs=4,
                            name=f"t{e}")
                nc.vector._custom_dve(
                    TSUB_OP, out=t[:], in0=e2nT[1][e][:], in1=pr[:],
                    s0=neg_e1T[1][e][:, b1:b1 + 1])
                u = mp.tile([128, B2], BF16, tag=f"u{e}", bufs=4,
                            name=f"u{e}")
                if e % 2 == 0:
                    nc.gpsimd.tensor_mul(u[:], Pg[e][:], t[:])
                else:
                    nc.vector.tensor_mul(u[:], Pg[e][:], t[:])
                nc.vector.tensor_add(nh[e][:], u[:], pr[:])
            nh_of[b1] = nh

        def emit_l2(b1, n_red):
            nh1 = nh_of.pop(b1)
            Pg = _alloc4(mp, "qg", BF16)
            for k in range(KT):
                nc.scalar.activation(
                    Pg[k][:], cert2T[2][k][:], AF.Sigmoid,
                    bias=betaT[k][:, 1:2],
                    scale=aC1T[2][k][:, b1:b1 + 1])
            prs = []
            for e in range(KT):
                pr = pp.tile([128, B2], F32, tag="prop", name="prop")
                for k in range(KT):
                    nc.tensor.matmul(
                        pr[:],
                        lhsT=Wp[2][k][:, e * 128:(e + 1) * 128],
                        rhs=nh1[k][:],
                        start=(k == 0), stop=(k == KT - 1))
                prs.append(pr)
            for e in range(KT):
                pr = prs[e]
                t = mp.tile([128, B2], BF16, tag=f"s{e}", bufs=4,
                            name=f"s{e}")
                nc.vector._custom_dve(
                    TSUB_OP, out=t[:], in0=e2nT[2][e][:], in1=pr[:],
                    s0=neg_e1T[2][e][:, b1:b1 + 1])
                u = mp.tile([128, B2], BF16, tag=f"v{e}", bufs=4,
                            name=f"v{e}")
                if e % 2 == 0:
                    nc.gpsimd.tensor_mul(u[:], Pg[e][:], t[:])
                else:
                    nc.vector.tensor_mul(u[:], Pg[e][:], t[:])
                nc.tensor.matmul(
                    ovr_ps[:],
                    lhsT=zcat[:, b1 * 32:(b1 + 1) * 32],
                    rhs=u[:],
                    start=(n_red == 0), stop=False,
                    skip_group_check=True)
                n_red += 1
            for k in range(KT):
                nc.tensor.matmul(
                    ovr_ps[:],
                    lhsT=w2stair[k][:, 31 - b1:63 - b1],
                    rhs=nh1[k][:],
                    start=False, stop=(n_red == N_RED_TOTAL - 1),
                    skip_group_check=True)
                n_red += 1
            return n_red

        n_red = 0
        SKEW = 2
        for b1 in range(B1C + SKEW):
            if b1 < B1C:
                emit_l0_l1(b1)
            if b1 >= SKEW:
                n_red = emit_l2(b1 - SKEW, n_red)

        ovr_sb = mp.tile([B1C, B2], F32, tag="ovr_sb", bufs=1)
        nc.vector.tensor_copy(ovr_sb[:], ovr_ps[:])
        nc.sync.dma_start(ovr_out[:, :], ovr_sb[:])


_CACHED = None


def _build():
    global _CACHED
    if _CACHED is not None:
        return _CACHED
    nc = bacc.Bacc(
        "TRN2", target_bir_lowering=False, debug=False,
        enable_asserts=False, num_devices=NCORES)
    io = (
        nc.dram_tensor("emb1", (L, B1C, D), F32, kind="ExternalInput").ap(),
        nc.dram_tensor("cert1", (L, B1C, D), F32, kind="ExternalInput").ap(),
        nc.dram_tensor("emb2", (L, B2, D), F32, kind="ExternalInput").ap(),
        nc.dram_tensor("cert2", (L, B2, D), F32, kind="ExternalInput").ap(),
        nc.dram_tensor("links", (L - 1, D, D), F32, kind="ExternalInput").ap(),
        nc.dram_tensor("alpha", (L - 1, D), F32, kind="ExternalInput").ap(),
        nc.dram_tensor("beta", (L - 1, D), F32, kind="ExternalInput").ap(),
        nc.dram_tensor("ovr", (B1C, B2), F32, kind="ExternalOutput").ap(),
    )
    with tile.TileContext(nc) as tc:
        build_kernel(tc, io)
    nc.compile()
    _CACHED = nc
    return nc


def _run(inputs, trace=False, **kw):
    nc = _build()
    arr = {k: np.ascontiguousarray(np.asarray(v, dtype=np.float32))
           for k, v in inputs.items()}
    in_maps = []
    for c in range(NCORES):
        sl = slice(c * B1C, (c + 1) * B1C)
        in_maps.append({
            "emb1": arr["emb1"][:, sl, :],
            "cert1": arr["cert1"][:, sl, :],
            "emb2": arr["emb2"],
            "cert2": arr["cert2"],
            "links": arr["links"],
            "alpha": arr["alpha"],
            "beta": arr["beta"],
        })
    res = bass_utils.run_bass_kernel_spmd(
        nc, in_maps, core_ids=list(range(NCORES)), trace=trace, **kw)
    out = np.concatenate([res.results[c]["ovr"] for c in range(NCORES)],
                         axis=0)
    return out, res


def kernel(**inputs) -> np.ndarray:
    try:
        out, _ = _run(inputs, trace=False)
    except Exception:
        # transient device failures (e.g. NRT_EXEC_UNIT_UNRECOVERABLE):
        # retry once on a fresh run
        import time
        time.sleep(5)
        out, _ = _run(inputs, trace=False)
    return out.astype(np.float32)


if __name__ == "__main__":
    nc = _build()
    print("build+compile OK; instructions:",
          sum(len(b.instructions) for f in nc.m.functions for b in f.blocks))


# revision 45
# speedup vs baseline: 1.1634x; 1.0061x over previous
"""AVSL similarity kernel for 8x trn2 NeuronCores (Bass/Tile).

Strategy
--------
Shard B1 (=256 query rows) across 8 cores, 32 rows each. All other inputs
are replicated. Everything else happens on-device per core:

Layout: "transposed" -- d (=512) on partitions (4 tiles of 128), B2 (=512)
on the free dim. This makes every per-(b1)-row quantity (e1n, alpha*cert1,
beta) a per-partition scalar, which ACT/DVE broadcast natively along the
free dim.

Per layer l:
  P^T    = Sigmoid(alpha*c1[b1] * c2^T + beta)   -- one ACT op/tile
  prop^T = W'^T_math @ nh^T                      -- PE, bf16, fp32 accum
  t      = (e2n^T - e1n[b1])^2 - prop            -- one fused custom DVE op
  nh^T   = P*t + prop                            -- mult (gpsimd/DVE) + add
Final: ovr[b1,:] = sum_e u2 + nh1 @ colsum(W2'): done on PE with M=32
one-hot selector weights so all 32 rows accumulate into one PSUM bank.

The main loop is software-pipelined with a 1-row skew: row b1's layer-1
matmuls are emitted alongside row (b1-1)'s layer-2 work so the PE never
waits on the elementwise tail of the current row.

Top-3 masking of links (per column) is done on-device in W^T layout via
3x reduce_max + scalar_tensor_tensor masking; colsum of the top-3 = m1+m2+m3.
"""

import sys

for _p in ("/opt/trn_rl_repo",):
    if _p not in sys.path:
        sys.path.insert(0, _p)

import numpy as np

import concourse.bass as bass
import concourse.bacc as bacc
import concourse.tile as tile
from concourse import mybir
from concourse import bass_utils
from concourse.masks import make_identity
import concourse.dve_ops as dve_ops
from concourse.dve_spec import Spec, Src0, Src1, C0, sq
from concourse.dve_table_gen import dve_ver_for


def _register_dve_op(name, spec):
    """Register a custom DVE op, self-pinning its uops sha."""
    import re
    for existing in dve_ops.OPS:
        if existing.name == name:
            return existing
    ver = dve_ver_for("TRN2")
    op = dve_ops.DveOp(name, spec, subdim=False, uops_sha={})
    dve_ops.OPS.append(op)
    dve_ops._SUB_OPCODE_FOR_NAME[name] = (
        dve_ops._CUSTOM_DVE_ROW_BASE + len(dve_ops.OPS) - 1)
    try:
        op.compile(ver)
    except ValueError as e:
        sha = re.search(r"v\d: ([0-9a-f]+)", str(e)).group(1)
        op = dve_ops.DveOp(name, spec, subdim=False, uops_sha={ver: sha})
        dve_ops.OPS[-1] = op
        dve_ops._COMPILE_CACHE.pop((name, ver), None)
    return op


# t = (e2n - e1n)^2 - prop in one DVE pass (in0=e2n, s0=-e1n, in1=prop)
TSUB_OP = _register_dve_op(
    "ANT_AVSL_TSUB",
    Spec(body=sq(Src0 + C0) - Src1,
         reference=lambda in0, in1, s0, s1, imm2:
             (in0.astype("float32") + s0) ** 2 - in1))

F32 = mybir.dt.float32
BF16 = mybir.dt.bfloat16
AF = mybir.ActivationFunctionType
ALU = mybir.AluOpType
AX = mybir.AxisListType.X

L = 3
B1 = 256
B2 = 512
D = 512
NCORES = 8
B1C = B1 // NCORES          # 32 rows per core
KT = D // 128               # 4 partition tiles of d (and of e, and of b2)
TOPK = 3
EPS_DIV = 1e-8


def _alloc4(pool, tag, dtype, shape=(128, B2), bufs=None):
    return [pool.tile(list(shape), dtype, tag=f"{tag}{k}", name=f"{tag}{k}",
                      bufs=bufs)
            for k in range(KT)]


def _transpose_tiles(nc, prep_ps, ident, nat_tiles, out_f=None, dt=F32):
    """Transpose natural tiles (p, 512) -> 4 tiles (128, p*n) via PE."""
    n_in = len(nat_tiles)
    p_in = nat_tiles[0].shape[0]
    for k in range(KT):
        ps = prep_ps.tile([128, p_in * n_in], dt, tag="tps", bufs=2,
                          name="tps")
        for j, nat in enumerate(nat_tiles):
            nc.tensor.transpose(
                ps[:, j * p_in:(j + 1) * p_in],
                nat[:, k * 128:(k + 1) * 128],
                ident[:p_in, :p_in],
            )
        out_f(k, ps)


def _rsqrt(nc, pool, ss, p, w=1):
    """r = 1/sqrt(ss), ss (p,w) f32. ACT sqrt seed + reciprocal + 2 Newton."""
    n = pool.tile([p, w], F32, tag="rs_n", name="rs_n")
    nc.scalar.activation(n[:], ss[:], AF.Sqrt)
    r = pool.tile([p, w], F32, tag="rs_r", name="rs_r")
    nc.vector.reciprocal(r[:], n[:])
    for it in range(1):
        a = pool.tile([p, w], F32, tag="rs_a", name="rs_a")
        nc.vector.tensor_mul(a[:], r[:], r[:])          # r^2
        b = pool.tile([p, w], F32, tag="rs_b", name="rs_b")
        nc.vector.tensor_mul(b[:], a[:], ss[:])         # ss*r^2
        c = pool.tile([p, w], F32, tag="rs_c", name="rs_c")
        nc.vector.tensor_scalar(c[:], b[:], -0.5, 1.5, ALU.mult, ALU.add)
        r2 = pool.tile([p, w], F32, tag="rs_r", name="rs_r")
        nc.vector.tensor_mul(r2[:], r[:], c[:])
        r = r2
    return r


def build_kernel(tc, io):
    nc = tc.nc
    emb1, cert1, emb2, cert2, links, alpha, beta, ovr_out = io

    import contextlib
    ctx = contextlib.ExitStack()
    with ctx:
        consts = ctx.enter_context(tc.tile_pool(name="consts", bufs=1))
        shared = ctx.enter_context(tc.tile_pool(name="shared", bufs=1))
        pctx = ctx.enter_context(contextlib.ExitStack())
        prep = pctx.enter_context(tc.tile_pool(name="prep", bufs=1))
        small = pctx.enter_context(tc.tile_pool(name="small", bufs=4))
        prep_ps = pctx.enter_context(
            tc.tile_pool(name="prep_ps", bufs=1, space="PSUM"))

        ident_f = consts.tile([128, 128], F32)
        make_identity(nc, ident_f[:])
        ident_b = consts.tile([128, 128], BF16)
        make_identity(nc, ident_b[:])

        # contiguous per-row selectors: zcat[:, b1*32:(b1+1)*32] has
        # one-hot column b1 (ones in every partition)
        zcat = consts.tile([128, 32 * B1C], BF16)
        nc.vector.memset(zcat[:], 0.0)
        for j in range(B1C):
            nc.gpsimd.affine_select(
                zcat[:, j * 32:(j + 1) * 32], zcat[:, j * 32:(j + 1) * 32],
                pattern=[[-1, 32]], compare_op=ALU.not_equal, fill=1.0,
                base=j, channel_multiplier=0)

        # ---------------- alpha / beta columns -----------------------------
        ab_nat = prep.tile([2, 512], F32, tag="abn")
        nc.sync.dma_start(ab_nat[:], alpha[:, :])
        bt_nat = prep.tile([2, 512], F32, tag="abn")
        nc.sync.dma_start(bt_nat[:], beta[:, :])
        alphaT = [consts.tile([128, 2], F32, tag=f"alT{k}", name=f"alT{k}")
                  for k in range(KT)]
        betaT = [consts.tile([128, 2], F32, tag=f"beT{k}", name=f"beT{k}")
                 for k in range(KT)]
        for src, dst in ((ab_nat, alphaT), (bt_nat, betaT)):
            for k in range(KT):
                ps = prep_ps.tile([128, 2], F32, tag="tps", bufs=2,
                                  name="tps")
                nc.tensor.transpose(
                    ps[:], src[:, k * 128:(k + 1) * 128], ident_f[:2, :2])
                nc.vector.tensor_copy(dst[k][:], ps[:])

        # ---------------- e2 normalize + transpose -------------------------
        e2nT = [[shared.tile([128, B2], BF16, tag=f"e2nT{l}{k}",
                             name=f"e2nT{l}{k}")
                 for k in range(KT)] for l in range(L)]
        for l in range(L):
            raw = []
            ss = small.tile([128, KT], F32, tag="ss", name="ss")
            for j in range(KT):
                nat = prep.tile([128, D], F32, tag=f"stg{j}",
                                name=f"stg{j}", bufs=2)
                nc.sync.dma_start(nat[:], emb2[l, j * 128:(j + 1) * 128, :])
                sqs = prep.tile([128, D], F32, tag="sqscratch",
                                name="sqscratch", bufs=1)
                nc.scalar.activation(sqs[:], nat[:], AF.Square,
                                     accum_out=ss[:, j:j + 1])
                raw.append(nat)
            r = _rsqrt(nc, small, ss, 128, KT)
            nats = []
            for j in range(KT):
                en = prep.tile([128, D], BF16, tag=f"e2n{j}",
                                name=f"e2n{j}", bufs=2)
                nc.vector.tensor_scalar_mul(en[:], raw[j][:], r[:, j:j + 1])
                nats.append(en)
            _transpose_tiles(
                nc, prep_ps, ident_b, nats, dt=BF16,
                out_f=lambda k, ps, l=l: nc.vector.tensor_copy(
                    e2nT[l][k][:], ps[:]))

        # ---------------- cert2 transpose (l=1,2) --------------------------
        cert2T = [None] + [[shared.tile([128, B2], BF16, tag=f"c2T{l}{k}",
                                        name=f"c2T{l}{k}")
                            for k in range(KT)] for l in (1, 2)]
        for l in (1, 2):
            nats = []
            for j in range(KT):
                nat = prep.tile([128, D], F32, tag=f"stg{j}",
                                name=f"stg{j}", bufs=2)
                nc.sync.dma_start(nat[:], cert2[l, j * 128:(j + 1) * 128, :])
                nats.append(nat)
            _transpose_tiles(
                nc, prep_ps, ident_f, nats,
                out_f=lambda k, ps, l=l: nc.vector.tensor_copy(
                    cert2T[l][k][:], ps[:]))

        # ---------------- e1 normalize + transpose (negated) ---------------
        neg_e1T = [[consts.tile([128, B1C], F32, tag=f"ne1T{l}{k}",
                                name=f"ne1T{l}{k}")
                    for k in range(KT)] for l in range(L)]
        for l in range(L):
            nat = prep.tile([B1C, D], F32, tag="e1nat")
            nc.sync.dma_start(nat[:], emb1[l, :, :])
            sqs = prep.tile([B1C, D], F32, tag="sq1scratch")
            ss = small.tile([B1C, 1], F32, tag="ss1")
            nc.scalar.activation(sqs[:], nat[:], AF.Square, accum_out=ss[:])
            r = _rsqrt(nc, small, ss, B1C)
            # fold the negation into the normalize scale: -e1n = nat * (-r)
            rneg = small.tile([B1C, 1], F32, tag="rneg")
            nc.vector.tensor_scalar_mul(rneg[:], r[:], -1.0)
            en = prep.tile([B1C, D], F32, tag="e1n")
            nc.vector.tensor_scalar_mul(en[:], nat[:], rneg[:])
            _transpose_tiles(
                nc, prep_ps, ident_f, [en],
                out_f=lambda k, ps, l=l: nc.vector.tensor_copy(
                    neg_e1T[l][k][:], ps[:]))

        # ---------------- cert1 -> alpha * cert1^T (l=1,2) -----------------
        aC1T = [None, [], []]
        for l in (1, 2):
            nat = prep.tile([B1C, D], F32, tag="c1nat")
            nc.sync.dma_start(nat[:], cert1[l, :, :])

            def mk(k, ps, l=l):
                t = consts.tile([128, B1C], F32, tag=f"ac1T{l}{k}",
                                name=f"ac1T{l}{k}")
                nc.vector.tensor_scalar_mul(t[:], ps[:], alphaT[k][:, l - 1:l])
                aC1T[l].append(t)
            _transpose_tiles(nc, prep_ps, ident_f, [nat], out_f=mk)

        # ---------------- links: topk mask + normalize + transpose ---------
        Wp = [None, [], []]   # W' natural, bf16, per layer: 4 tiles (128d, 512e)
        w2stair = []          # selector tiles carrying colsum(W2') at col b1
        for l in (1, 2):
            nats = []
            for j in range(KT):
                nat = prep.tile([128, D], F32, tag=f"stg{j}",
                                name=f"stg{j}", bufs=2)
                nc.sync.dma_start(nat[:], links[l - 1, j * 128:(j + 1) * 128, :])
                nats.append(nat)
            WT = []

            def stash(k, ps):
                t = prep.tile([128, D], F32, tag=f"wT{k}", name=f"wT{k}")
                nc.vector.tensor_copy(t[:], ps[:])
                WT.append(t)
            _transpose_tiles(nc, prep_ps, ident_f, nats, out_f=stash)

            WpT = []
            for k in range(KT):
                m1 = small.tile([128, 1], F32, tag="m1")
                nc.vector.tensor_reduce(m1[:], WT[k][:], axis=AX, op=ALU.max)
                w1 = prep.tile([128, D], F32, tag="wtmp", name="w1", bufs=2)
                nc.vector.scalar_tensor_tensor(
                    w1[:], WT[k][:], m1[:], WT[k][:], ALU.is_lt, ALU.mult)
                m2 = small.tile([128, 1], F32, tag="m2")
                nc.vector.tensor_reduce(m2[:], w1[:], axis=AX, op=ALU.max)
                w2 = prep.tile([128, D], F32, tag="wtmp", name="w2", bufs=2)
                nc.vector.scalar_tensor_tensor(
                    w2[:], w1[:], m2[:], w1[:], ALU.is_lt, ALU.mult)
                m3 = small.tile([128, 1], F32, tag="m3")
                nc.vector.tensor_reduce(m3[:], w2[:], axis=AX, op=ALU.max)
                # colsum of kept top-3 = m1+m2+m3 (+eps); scale = 1/that
                cs = small.tile([128, 1], F32, tag="cs")
                nc.vector.tensor_add(cs[:], m1[:], m2[:])
                cs2 = small.tile([128, 1], F32, tag="cs2")
                nc.vector.tensor_add(cs2[:], cs[:], m3[:])
                cs3 = small.tile([128, 1], F32, tag="cs3")
                nc.vector.tensor_scalar_add(cs3[:], cs2[:], EPS_DIV)
                rcs = small.tile([128, 1], F32, tag="rcs")
                nc.vector.reciprocal(rcs[:], cs3[:])
                wm = prep.tile([128, D], F32, tag="wtmp", name="wm", bufs=2)
                nc.vector.scalar_tensor_tensor(
                    wm[:], WT[k][:], m3[:], WT[k][:], ALU.is_ge, ALU.mult)
                wpt = prep.tile([128, D], BF16, tag=f"wpT{k}", name=f"wpT{k}")
                nc.vector.tensor_scalar_mul(wpt[:], wm[:], rcs[:])
                WpT.append(wpt)

            # transpose back to natural (d on partitions) in bf16
            for j in range(KT):
                ps = prep_ps.tile([128, D], BF16, tag="tps", bufs=2,
                                  name="tps")
                for k in range(KT):
                    nc.tensor.transpose(
                        ps[:, k * 128:(k + 1) * 128],
                        WpT[k][:, j * 128:(j + 1) * 128],
                        ident_b[:])
                wn = shared.tile([128, D], BF16, tag=f"wp{l}{j}",
                                 name=f"wp{l}{j}")
                nc.vector.tensor_copy(wn[:], ps[:])
                Wp[l].append(wn)

            if l == 2:
                for j in range(KT):
                    w2s = small.tile([128, 1], F32, tag="w2s")
                    nc.vector.tensor_reduce(
                        w2s[:], Wp[2][j][:], axis=AX, op=ALU.add)
                    st = consts.tile([128, 63], BF16, tag=f"w2st{j}",
                                     name=f"w2st{j}")
                    nc.vector.memset(st[:], 0.0)
                    nc.vector.tensor_copy(st[:, 31:32], w2s[:])
                    w2stair.append(st)

        # ======================= main loop over b1 ==========================
        # Software-pipelined: emit (b1, layer1) and (b1-1, layer2) together.
        mp = ctx.enter_context(tc.tile_pool(name="main", bufs=3))
        pp = ctx.enter_context(tc.tile_pool(name="mps", bufs=5, space="PSUM"))
        pov = ctx.enter_context(tc.tile_pool(name="ovrps", bufs=1,
                                             space="PSUM"))
        ovr_ps = pov.tile([B1C, B2], F32, tag="ovr")
        N_RED_TOTAL = B1C * 2 * KT
        nh_of = {}

        def emit_l0_l1(b1):
            nh0 = _alloc4(mp, "n0", BF16)
            for k in range(KT):
                nc.scalar.activation(
                    nh0[k][:], e2nT[0][k][:], AF.Square,
                    bias=neg_e1T[0][k][:, b1:b1 + 1])
            Pg = _alloc4(mp, "pg", BF16)
            for k in range(KT):
                nc.scalar.activation(
                    Pg[k][:], cert2T[1][k][:], AF.Sigmoid,
                    bias=betaT[k][:, 0:1],
                    scale=aC1T[1][k][:, b1:b1 + 1])
            nh = _alloc4(mp, "nh", BF16, bufs=4)
            prs = []
            for e in range(KT):
                pr = pp.tile([128, B2], F32, tag="prop", name="prop")
                for k in range(KT):
                    nc.tensor.matmul(
                        pr[:],
                        lhsT=Wp[1][k][:, e * 128:(e + 1) * 128],
                        rhs=nh0[k][:],
                        start=(k == 0), stop=(k == KT - 1))
                prs.append(pr)
            for e in range(KT):
                pr = prs[e]
                t = mp.tile([128, B2], BF16, tag=f"t{e}", bufs=3,
                            name=f"t{e}")
                nc.vector._custom_dve(
                    TSUB_OP, out=t[:], in0=e2nT[1][e][:], in1=pr[:],
                    s0=neg_e1T[1][e][:, b1:b1 + 1])
                u = mp.tile([128, B2], BF16, tag=f"u{e}", bufs=3,
                            name=f"u{e}")
                if e % 2 == 0:
                    nc.gpsimd.tensor_mul(u[:], Pg[e][:], t[:])
                else:
                    nc.vector.tensor_mul(u[:], Pg[e][:], t[:])
                nc.vector.tensor_add(nh[e][:], u[:], pr[:])
            nh_of[b1] = nh

        def emit_l2(b1, n_red):
            nh1 = nh_of.pop(b1)
            Pg = _alloc4(mp, "qg", BF16)
            for k in range(KT):
                nc.scalar.activation(
                    Pg[k][:], cert2T[2][k][:], AF.Sigmoid,
                    bias=betaT[k][:, 1:2],
                    scale=aC1T[2][k][:, b1:b1 + 1])
            prs = []
            for e in range(KT):
                pr = pp.tile([128, B2], F32, tag="prop", name="prop")
                for k in range(KT):
                    nc.tensor.matmul(
                        pr[:],
                        lhsT=Wp[2][k][:, e * 128:(e + 1) * 128],
                        rhs=nh1[k][:],
                        start=(k == 0), stop=(k == KT - 1))
                prs.append(pr)
            for e in range(KT):
                pr = prs[e]
                t = mp.tile([128, B2], BF16, tag=f"s{e}", bufs=3,
                            name=f"s{e}")
                nc.vector._custom_dve(
                    TSUB_OP, out=t[:], in0=e2nT[2][e][:], in1=pr[:],
                    s0=neg_e1T[2][e][:, b1:b1 + 1])
                u = mp.tile([128, B2], BF16, tag=f"v{e}", bufs=3,
                            name=f"v{e}")
                if e % 2 == 0:
                    nc.gpsimd.tensor_mul(u[:], Pg[e][:], t[:])
                else:
                    nc.vector.tensor_mul(u[:], Pg[e][:], t[:])
                nc.tensor.matmul(
                    ovr_ps[:],
                    lhsT=zcat[:, b1 * 32:(b1 + 1) * 32],
                    rhs=u[:],
                    start=(n_red == 0), stop=False,
                    skip_group_check=True)
                n_red += 1
            for k in range(KT):
                nc.tensor.matmul(
                    ovr_ps[:],
                    lhsT=w2stair[k][:, 31 - b1:63 - b1],
                    rhs=nh1[k][:],
                    start=False, stop=(n_red == N_RED_TOTAL - 1),
                    skip_group_check=True)
                n_red += 1
            return n_red

        n_red = 0
        SKEW = 2
        for b1 in range(B1C + SKEW):
            if b1 < B1C:
                emit_l0_l1(b1)
            if b1 >= SKEW:
                n_red = emit_l2(b1 - SKEW, n_red)

        ovr_sb = mp.tile([B1C, B2], F32, tag="ovr_sb", bufs=1)
        nc.vector.tensor_copy(ovr_sb[:], ovr_ps[:])
        nc.sync.dma_start(ovr_out[:, :], ovr_sb[:])


_CACHED = None


def _build():
    global _CACHED
    if _CACHED is not None:
        return _CACHED
    nc = bacc.Bacc(
        "TRN2", target_bir_lowering=False, debug=False,
        enable_asserts=False, num_devices=NCORES)
    io = (
        nc.dram_tensor("emb1", (L, B1C, D), F32, kind="ExternalInput").ap(),
        nc.dram_tensor("cert1", (L, B1C, D), F32, kind="ExternalInput").ap(),
        nc.dram_tensor("emb2", (L, B2, D), F32, kind="ExternalInput").ap(),
        nc.dram_tensor("cert2", (L, B2, D), F32, kind="ExternalInput").ap(),
        nc.dram_tensor("links", (L - 1, D, D), F32, kind="ExternalInput").ap(),
        nc.dram_tensor("alpha", (L - 1, D), F32, kind="ExternalInput").ap(),
        nc.dram_tensor("beta", (L - 1, D), F32, kind="ExternalInput").ap(),
        nc.dram_tensor("ovr", (B1C, B2), F32, kind="ExternalOutput").ap(),
    )
    with tile.TileContext(nc) as tc:
        build_kernel(tc, io)
    nc.compile()
    _CACHED = nc
    return nc


def _run(inputs, trace=False, **kw):
    nc = _build()
    arr = {k: np.ascontiguousarray(np.asarray(v, dtype=np.float32))
           for k, v in inputs.items()}
    in_maps = []
    for c in range(NCORES):
        sl = slice(c * B1C, (c + 1) * B1C)
        in_maps.append({
            "emb1": arr["emb1"][:, sl, :],
            "cert1": arr["cert1"][:, sl, :],
            "emb2": arr["emb2"],
            "cert2": arr["cert2"],
            "links": arr["links"],
            "alpha": arr["alpha"],
            "beta": arr["beta"],
        })
    res = bass_utils.run_bass_kernel_spmd(
        nc, in_maps, core_ids=list(range(NCORES)), trace=trace, **kw)
    out = np.concatenate([res.results[c]["ovr"] for c in range(NCORES)],
                         axis=0)
    return out, res


def kernel(**inputs) -> np.ndarray:
    try:
        out, _ = _run(inputs, trace=False)
    except Exception:
        # transient device failures (e.g. NRT_EXEC_UNIT_UNRECOVERABLE):
        # retry once on a fresh run
        import time
        time.sleep(5)
        out, _ = _run(inputs, trace=False)
    return out.astype(np.float32)


if __name__ == "__main__":
    nc = _build()
    print("build+compile OK; instructions:",
          sum(len(b.instructions) for f in nc.m.functions for b in f.blocks))
